# Initial kernel scaffold
#
"""Distributed Trainium2 Bass kernel for the AMK transformer block.

Sharding (8 NeuronCores):
  Stage 1 (head tensor-parallel): core c owns heads {2c, 2c+1}. It computes the
  qkv projection for its head slice over all 4096 tokens, RoPE + per-head
  RMS-norm on q/k, non-causal attention (scores transposed so the exp'd
  probability tiles feed the PV matmul directly; softmax denominators come from
  a ones-column appended to V), m = C - V, and a partial o-projection.
  Stage 2: two chunked ReduceScatters (one per batch) hand each core a
  256-token slice per chunk with the fully-summed attention output; the
  orthogonal correction, both RMS norms and the full SwiGLU MLP (weights
  replicated) run token-sharded, overlapping the second RS with the first
  MLP half.  Output shards are concatenated host-side.

Matmuls run in bf16 (4x the fp32 PE rate); all accumulation, softmax and
norm glue stays fp32.  RMS-normed q/k bound scores to [-8, 8], so softmax
needs no max subtraction.
"""
import math
import os
import sys

import numpy as np

try:
    import concourse  # noqa: F401
except ImportError:  # grading env: same container layout
    sys.path.insert(0, "/opt/trn_rl_repo")

import ml_dtypes

import concourse.bass as bass
import concourse.tile as tile
from concourse import mybir
from concourse.bass_utils import run_bass_kernel_spmd

BF16 = mybir.dt.bfloat16
F32 = mybir.dt.float32
AF = mybir.ActivationFunctionType
ALU = mybir.AluOpType
AX = mybir.AxisListType

B, N, D = 2, 2048, 1024
H, DH = 16, 64
INTER = 2816
EPS = 1e-5
T = B * N            # 4096
NC = 8
HC = H // NC         # heads per core
DHC = HC * DH        # 128
CHUNK = 256          # tokens per core per RS chunk
SHARD = B * CHUNK    # 512 tokens per core total
NB = 512             # q-token block in attention
MT = N // 128        # m-tiles per batch (16)


def _build():
    nc = bass.Bass()

    hsT = nc.declare_dram_parameter("hsT", [D, T], BF16, isOutput=False)
    wqkvT = nc.declare_dram_parameter("wqkvT", [D, 3 * DHC], BF16, isOutput=False)
    cosT2 = nc.declare_dram_parameter("cosT2", [DHC, N], F32, isOutput=False)
    sinT2 = nc.declare_dram_parameter("sinT2", [DHC, N], F32, isOutput=False)
    woT = nc.declare_dram_parameter("woT", [DHC, D], BF16, isOutput=False)
    hs_sh = nc.declare_dram_parameter("hs_sh", [SHARD, D], F32, isOutput=False)
    wguT = nc.declare_dram_parameter("wguT", [D, 2 * INTER], BF16, isOutput=False)
    wdT = nc.declare_dram_parameter("wdT", [INTER, D], BF16, isOutput=False)
    ident = nc.declare_dram_parameter("ident", [128, 128], BF16, isOutput=False)
    ones64 = nc.declare_dram_parameter("ones64", [1, 64], BF16, isOutput=False)
    blkA = nc.declare_dram_parameter("blkA", [128, 2], BF16, isOutput=False)
    blkB = nc.declare_dram_parameter("blkB", [2, 128], BF16, isOutput=False)
    out_ext = nc.declare_dram_parameter("out", [SHARD, D], F32, isOutput=True)

    # collective bounce buffers (internal DRAM)
    rs_in = [nc.dram_tensor(f"rs_in{k}", [N, D], F32) for k in range(B)]
    rs_out = [nc.dram_tensor(f"rs_out{k}", [CHUNK, D], F32, addr_space="Shared")
              for k in range(B)]

    with tile.TileContext(nc, num_cores=NC) as tc:
        with (
            tc.tile_pool(name="big", bufs=1) as big,        # long-lived big tensors
            tc.tile_pool(name="wts", bufs=1) as wts,
            tc.tile_pool(name="cons", bufs=1) as cons,
            tc.tile_pool(name="work", bufs=3) as work,
            tc.tile_pool(name="pt", bufs=3) as pt,
            tc.tile_pool(name="ps", bufs=2, space="PSUM") as ps,
            tc.tile_pool(name="psc", bufs=2, space="PSUM") as psc,
            tc.tile_pool(name="pss", bufs=2, space="PSUM") as pss,
            tc.tile_pool(name="pso", bufs=2, space="PSUM") as pso,
        ):
            # ---- constant loads ----
            c_id = cons.tile([128, 128], BF16, tag="ident")
            nc.sync.dma_start(c_id[:], ident[:])
            c_ones64 = cons.tile([1, 64], BF16, tag="ones64")
            nc.sync.dma_start(c_ones64[:], ones64[:])
            c_blkA = cons.tile([128, 2], BF16, tag="blkA")
            nc.sync.dma_start(c_blkA[:], blkA[:])
            c_blkB = cons.tile([2, 128], BF16, tag="blkB")
            nc.sync.dma_start(c_blkB[:], blkB[:])
            c_cos = cons.tile([DHC, N], F32, tag="cos")
            nc.sync.dma_start(c_cos[:], cosT2[:])
            c_sin = cons.tile([DHC, N], F32, tag="sin")
            nc.sync.dma_start(c_sin[:], sinT2[:])

            # ---- phase A: qkv projection over all tokens ----
            sb_hsT = big.tile([D, T], BF16, tag="hsT")      # 8 MB
            nc.sync.dma_start(sb_hsT[:], hsT[:])
            sb_wqkv = cons.tile([D, 3 * DHC], BF16, tag="wqkv")
            nc.sync.dma_start(sb_wqkv[:], wqkvT[:])

            # outputs of phase A
            sb_qh = big.tile([DHC, T], BF16, tag="qh")      # normed q, 1 MB
            sb_kh = big.tile([DHC, T], BF16, tag="kh")
            sb_vT = big.tile([DHC, T], BF16, tag="vT")      # feature-major v
            # token-major V with ones column: per (b, h2, mtile) a [128, 65]
            sb_va = big.tile([128, B * HC * MT * 65], BF16, tag="vaug")
            nc.vector.memset(sb_va[:], 1.0)  # ones col pre-set (rest overwritten)

            def vaug_slice(b, h2, mt):
                off = ((b * HC + h2) * MT + mt) * 65
                return sb_va[:, off:off + 65]

            for ot in range(3):          # 0=q, 1=k, 2=v
                for tb in range(T // 512):
                    pq = ps.tile([128, 512], F32, tag="qkv")
                    for dd in range(D // 128):
                        nc.tensor.matmul(
                            pq[:],
                            sb_wqkv[dd * 128:(dd + 1) * 128,
                                    ot * 128:(ot + 1) * 128],
                            sb_hsT[dd * 128:(dd + 1) * 128,
                                   tb * 512:(tb + 1) * 512],
                            start=(dd == 0), stop=(dd == D // 128 - 1),
                        )
                    col = (tb * 512) % N   # position within batch for cos/sin
                    if ot < 2:
                        # RoPE: t2 = swapped-rows * sin~ ; out = q*cos + t2
                        t2 = work.tile([128, 512], F32, tag="rope_t2")
                        for (dst, src) in ((0, 32), (32, 0), (64, 96), (96, 64)):
                            nc.vector.tensor_tensor(
                                t2[dst:dst + 32, :],
                                pq[src:src + 32, :],
                                c_sin[dst:dst + 32, col:col + 512],
                                ALU.mult,
                            )
                        t1 = work.tile([128, 512], F32, tag="rope_t1")
                        nc.vector.tensor_tensor(
                            t1[:], pq[:], c_cos[:, col:col + 512], ALU.mult)
                        tq = work.tile([128, 512], F32, tag="rope_q")
                        nc.vector.tensor_tensor(tq[:], t1[:], t2[:], ALU.add)
                        # per-head rms over the 64-row blocks
                        sq = work.tile([128, 512], BF16, tag="rope_sq")
                        nc.vector.tensor_tensor(sq[:], tq[:], tq[:], ALU.mult)
                        pr = psc.tile([2, 512], F32, tag="rms")
                        nc.tensor.matmul(pr[:], c_blkA[:], sq[:],
                                         start=True, stop=True)
                        std = work.tile([2, 512], F32, tag="rms_std")
                        if ot == 0:
                            # 8*sqrt(ssq/64+eps) = sqrt(ssq + 64*eps)
                            nc.scalar.activation(std[:], pr[:], AF.Sqrt,
                                                 bias=64.0 * EPS, scale=1.0)
                        else:
                            nc.scalar.activation(std[:], pr[:], AF.Sqrt,
                                                 bias=EPS, scale=1.0 / 64.0)
                        rin = work.tile([2, 512], F32, tag="rms_rin")
                        nc.vector.reciprocal(rin[:], std[:])
                        rinb = work.tile([2, 512], BF16, tag="rms_rinb")
                        nc.vector.tensor_copy(rinb[:], rin[:])
                        pb = psc.tile([128, 512], F32, tag="bcast")
                        nc.tensor.matmul(pb[:], c_blkB[:], rinb[:],
                                         start=True, stop=True)
                        dst = sb_qh if ot == 0 else sb_kh
                        nc.vector.tensor_tensor(
                            dst[:, tb * 512:(tb + 1) * 512],
                            tq[:], pb[:], ALU.mult)
                    else:
                        nc.vector.tensor_copy(
                            sb_vT[:, tb * 512:(tb + 1) * 512], pq[:])
                        # transpose v into token-major V_aug tiles
                        b = (tb * 512) // N
                        for h2 in range(HC):
                            for j in range(4):
                                mt = (col + j * 128) // 128
                                ptr = psc.tile([128, 64], F32, tag="vtr")
                                nc.tensor.transpose(
                                    ptr[:],
                                    sb_vT[h2 * 64:(h2 + 1) * 64,
                                          tb * 512 + j * 128:
                                          tb * 512 + (j + 1) * 128],
                                    c_id[0:64, 0:64],
                                )
                                nc.vector.tensor_copy(
                                    vaug_slice(b, h2, mt)[:, 0:64], ptr[:])

            # ---- MLP weights prefetch (after hsT no longer needed) ----
            sb_wgu = wts.tile([D, 2 * INTER], BF16, tag="wgu")   # 11 MB
            nc.sync.dma_start(sb_wgu[:], wguT[:])
            sb_wd = wts.tile([INTER, D], BF16, tag="wd")         # 5.5 MB
            nc.sync.dma_start(sb_wd[:], wdT[:])
            sb_wo = cons.tile([DHC, D], BF16, tag="wo")
            nc.sync.dma_start(sb_wo[:], woT[:])

            # ---- attention + partial o-proj, per batch ----
            for b in range(B):
                for nb in range(N // NB):
                    n0 = b * N + nb * NB          # global token col
                    mTt = pt.tile([128, NB], BF16, tag="mT")
                    for h2 in range(HC):
                        rows = slice(h2 * 64, (h2 + 1) * 64)
                        pc = psc.tile([65, NB], F32, tag="cacc")
                        for mt in range(MT):
                            sps = pss.tile([128, NB], F32, tag="scores")
                            nc.tensor.matmul(
                                sps[:],
                                sb_kh[rows, b * N + mt * 128:
                                      b * N + (mt + 1) * 128],
                                sb_qh[rows, n0:n0 + NB],
                                start=True, stop=True,
                            )
                            pexp = pt.tile([128, NB], BF16, tag="pexp")
                            nc.scalar.activation(pexp[:], sps[:], AF.Exp)
                            nc.tensor.matmul(
                                pc[:], vaug_slice(b, h2, mt), pexp[:],
                                start=(mt == 0), stop=(mt == MT - 1),
                            )
                        rinv = work.tile([1, NB], F32, tag="arinv")
                        nc.vector.reciprocal(rinv[:], pc[64:65, :])
                        rb = work.tile([1, NB], BF16, tag="arb")
                        nc.vector.tensor_copy(rb[:], rinv[:])
                        pbc = pss.tile([64, NB], F32, tag="abc")
                        nc.tensor.matmul(pbc[:], c_ones64[:], rb[:],
                                         start=True, stop=True)
                        ct = work.tile([64, NB], F32, tag="cnorm")
                        nc.vector.tensor_tensor(ct[:], pc[0:64, :], pbc[:],
                                                ALU.mult)
                        nc.vector.tensor_tensor(
                            mTt[rows, :], ct[:],
                            sb_vT[rows, n0:n0 + NB], ALU.subtract)
                    # partial o-proj for these NB tokens -> rs_in (token-major)
                    for j in range(NB // 128):
                        att = work.tile([128, D], F32, tag="attn_sb")
                        for oo in range(D // 512):
                            po = pso.tile([128, 512], F32, tag="oproj")
                            nc.tensor.matmul(
                                po[:],
                                mTt[:, j * 128:(j + 1) * 128],
                                sb_wo[:, oo * 512:(oo + 1) * 512],
                                start=True, stop=True,
                            )
                            nc.scalar.copy(att[:, oo * 512:(oo + 1) * 512],
                                           po[:])
                        nc.sync.dma_start(
                            rs_in[b][nb * NB + j * 128:
                                     nb * NB + (j + 1) * 128, :],
                            att[:])

                # ---- chunked ReduceScatter for this batch ----
                nc.gpsimd.collective_compute(
                    "ReduceScatter",
                    ALU.add,
                    ins=[rs_in[b].ap().opt()],
                    outs=[rs_out[b].ap().opt()],
                    replica_groups=[list(range(NC))],
                )

            # ---- stage 2: token-sharded glue + MLP, per chunk ----
            for b in range(B):
                for tt in range(CHUNK // 128):
                    row0 = b * CHUNK + tt * 128   # row in hs_sh / out
                    a_t = work.tile([128, D], F32, tag="s2_attn")
                    nc.sync.dma_start(a_t[:], rs_out[b][tt * 128:(tt + 1) * 128, :])
                    x_t = work.tile([128, D], F32, tag="s2_hs")
                    nc.sync.dma_start(x_t[:], hs_sh[row0:row0 + 128, :])
                    prod = work.tile([128, D], F32, tag="s2_prod")
                    dot = work.tile([128, 1], F32, tag="s2_dot")
                    nc.vector.tensor_tensor_reduce(
                        prod[:], a_t[:], x_t[:], 1.0, 0.0,
                        ALU.mult, ALU.add, dot[:])
                    sc = work.tile([128, 1], F32, tag="s2_sc")
                    nc.vector.tensor_scalar(sc[:], dot[:], -1.0, 1.0,
                                            ALU.mult, ALU.add)
                    htil = work.tile([128, D], F32, tag="s2_htil")
                    nc.vector.scalar_tensor_tensor(
                        htil[:], x_t[:], sc[:], a_t[:], ALU.mult, ALU.add)
                    ssq = work.tile([128, 1], F32, tag="s2_ssq")
                    nc.vector.tensor_tensor_reduce(
                        prod[:], htil[:], htil[:], 1.0, 0.0,
                        ALU.mult, ALU.add, ssq[:])
                    std = work.tile([128, 1], F32, tag="s2_std")
                    nc.scalar.activation(std[:], ssq[:], AF.Sqrt,
                                         bias=EPS, scale=1.0 / D)
                    rstd = work.tile([128, 1], F32, tag="s2_rstd")
                    nc.vector.reciprocal(rstd[:], std[:])
                    h_f = work.tile([128, D], F32, tag="s2_h")
                    nc.vector.tensor_scalar(h_f[:], htil[:], rstd[:], None,
                                            ALU.mult)
                    h_b = work.tile([128, D], BF16, tag="s2_hb")
                    nc.vector.tensor_copy(h_b[:], h_f[:])
                    # transpose h -> hT (feature-major) for gate_up rhs
                    hT = pt.tile([128, 8 * 128], BF16, tag="s2_hT")
                    for dd in range(D // 128):
                        ptr = psc.tile([128, 128], F32, tag="s2_htr")
                        nc.tensor.transpose(
                            ptr[:], h_b[:, dd * 128:(dd + 1) * 128], c_id[:])
                        nc.vector.tensor_copy(
                            hT[:, dd * 128:(dd + 1) * 128], ptr[:])

                    # gate_up: feature-major [2I, 128]; silu(gate)*up -> actT
                    actT = pt.tile([128, INTER], BF16, tag="s2_actT")

                    def hT_chunk(dd):
                        return hT[:, dd * 128:(dd + 1) * 128]

                    for it in range(INTER // 128):    # 22 gate tiles
                        pg = ps.tile([128, 128], F32, tag="s2_gate")
                        pu = ps.tile([128, 128], F32, tag="s2_up")
                        for dd in range(D // 128):
                            nc.tensor.matmul(
                                pg[:],
                                sb_wgu[dd * 128:(dd + 1) * 128,
                                       it * 128:(it + 1) * 128],
                                hT_chunk(dd),
                                start=(dd == 0), stop=(dd == D // 128 - 1))
                        for dd in range(D // 128):
                            nc.tensor.matmul(
                                pu[:],
                                sb_wgu[dd * 128:(dd + 1) * 128,
                                       INTER + it * 128:INTER + (it + 1) * 128],
                                hT_chunk(dd),
                                start=(dd == 0), stop=(dd == D // 128 - 1))
                        sg = work.tile([128, 128], F32, tag="s2_silu")
                        nc.scalar.activation(sg[:], pg[:], AF.Silu)
                        nc.vector.tensor_tensor(
                            actT[:, it * 128:(it + 1) * 128]
                            .rearrange("p f -> p f"),
                            sg[:], pu[:], ALU.mult)

                    # down: token-major out [128, D]
                    pm0 = pso.tile([128, 512], F32, tag="s2_dn0")
                    pm1 = pso.tile([128, 512], F32, tag="s2_dn1")
                    for it in range(INTER // 128):
                        nc.tensor.matmul(
                            pm0[:], actT[:, it * 128:(it + 1) * 128],
                            sb_wd[it * 128:(it + 1) * 128, 0:512],
                            start=(it == 0), stop=(it == INTER // 128 - 1))
                        nc.tensor.matmul(
                            pm1[:], actT[:, it * 128:(it + 1) * 128],
                            sb_wd[it * 128:(it + 1) * 128, 512:1024],
                            start=(it == 0), stop=(it == INTER // 128 - 1))

                    o1 = work.tile([128, D], F32, tag="s2_o1")
                    nc.vector.tensor_tensor(o1[:, 0:512], h_f[:, 0:512],
                                            pm0[:], ALU.add)
                    nc.vector.tensor_tensor(o1[:, 512:1024], h_f[:, 512:1024],
                                            pm1[:], ALU.add)
                    ssq2 = work.tile([128, 1], F32, tag="s2_ssq2")
                    nc.vector.tensor_tensor_reduce(
                        prod[:], o1[:], o1[:], 1.0, 0.0,
                        ALU.mult, ALU.add, ssq2[:])
                    std2 = work.tile([128, 1], F32, tag="s2_std2")
                    nc.scalar.activation(std2[:], ssq2[:], AF.Sqrt,
                                         bias=EPS, scale=1.0 / D)
                    rstd2 = work.tile([128, 1], F32, tag="s2_rstd2")
                    nc.vector.reciprocal(rstd2[:], std2[:])
                    o_f = work.tile([128, D], F32, tag="s2_out")
                    nc.vector.tensor_scalar(o_f[:], o1[:], rstd2[:], None,
                                            ALU.mult)
                    nc.sync.dma_start(out_ext[row0:row0 + 128, :], o_f[:])

    return nc


_CACHE = {}


def _prep_inputs(inputs):
    hs = np.ascontiguousarray(inputs["hidden_states"], np.float32)
    cos = np.ascontiguousarray(inputs["cos"], np.float32)
    sin = np.ascontiguousarray(inputs["sin"], np.float32)
    w_qkv = np.ascontiguousarray(inputs["w_qkv"], np.float32)
    w_o = np.ascontiguousarray(inputs["w_o"], np.float32)
    w_gu = np.ascontiguousarray(inputs["w_gate_up"], np.float32)
    w_dn = np.ascontiguousarray(inputs["w_down"], np.float32)

    bf = ml_dtypes.bfloat16
    hs_flat = hs.reshape(T, D)
    hsT = np.ascontiguousarray(hs_flat.T).astype(bf)

    cosT = cos.T
    sinTn = sin.T.copy()
    sinTn[:32] *= -1.0
    cosT2 = np.ascontiguousarray(np.concatenate([cosT, cosT], 0), np.float32)
    sinT2 = np.ascontiguousarray(np.concatenate([sinTn, sinTn], 0), np.float32)

    ident = np.eye(128, dtype=bf)
    ones64 = np.ones((1, 64), dtype=bf)
    blkA = np.zeros((128, 2), dtype=bf)
    blkA[0:64, 0] = 1
    blkA[64:128, 1] = 1
    blkB = np.zeros((2, 128), dtype=bf)
    blkB[0, 0:64] = 1
    blkB[1, 64:128] = 1

    wguT = np.ascontiguousarray(w_gu.T).astype(bf)
    wdT = np.ascontiguousarray(w_dn.T).astype(bf)

    in_maps = []
    for c in range(NC):
        r0 = c * DHC
        wq = w_qkv[r0:r0 + DHC]
        wk = w_qkv[D + r0:D + r0 + DHC]
        wv = w_qkv[2 * D + r0:2 * D + r0 + DHC]
        wqkvT = np.ascontiguousarray(
            np.concatenate([wq, wk, wv], 0).T).astype(bf)
        woT = np.ascontiguousarray(w_o[:, r0:r0 + DHC].T).astype(bf)
        tok = np.concatenate([np.arange(CHUNK * c, CHUNK * (c + 1)),
                              N + np.arange(CHUNK * c, CHUNK * (c + 1))])
        in_maps.append({
            "hsT": hsT, "wqkvT": wqkvT, "cosT2": cosT2, "sinT2": sinT2,
            "woT": woT, "hs_sh": np.ascontiguousarray(hs_flat[tok]),
            "wguT": wguT, "wdT": wdT, "ident": ident, "ones64": ones64,
            "blkA": blkA, "blkB": blkB,
        })
    return in_maps


def kernel(**inputs):
    if "nc" not in _CACHE:
        _CACHE["nc"] = _build()
    nc = _CACHE["nc"]
    in_maps = _prep_inputs(inputs)
    res = run_bass_kernel_spmd(nc, in_maps, core_ids=list(range(NC)))
    out_full = np.empty((T, D), np.float32)
    for c in range(NC):
        shard = res.results[c]["out"]
        out_full[CHUNK * c:CHUNK * (c + 1)] = shard[:CHUNK]
        out_full[N + CHUNK * c:N + CHUNK * (c + 1)] = shard[CHUNK:]
    return out_full.reshape(B, N, D)


if __name__ == "__main__":
    rng = np.random.default_rng(0)
    fake = {
        "hidden_states": rng.standard_normal((B, N, D), np.float32),
        "cos": rng.random((N, DH), np.float32),
        "sin": rng.random((N, DH), np.float32),
        "w_qkv": rng.standard_normal((3 * D, D), np.float32) * 0.02,
        "w_o": rng.standard_normal((D, D), np.float32) * 0.02,
        "w_gate_up": rng.standard_normal((2 * INTER, D), np.float32) * 0.02,
        "w_down": rng.standard_normal((D, INTER), np.float32) * 0.02,
    }
    out = kernel(**fake)
    print("kernel ran, out shape", out.shape, out.dtype)


# revision 22
# speedup vs baseline: 1.1837x; 1.1837x over previous
"""Distributed Trainium2 Bass kernel for the AMK transformer block (8 cores).

Sharding:
  Stage 1 (head tensor-parallel): core c owns heads {2c, 2c+1}: qkv projection
  (float32r) for its head slice over all 4096 tokens, RoPE + per-head RMS on
  q/k folded into the score scale, non-causal attention with transposed score
  tiles (softmax denominators via a ones-column appended to V, no max
  subtraction needed since |scores| <= 8), m = C - V, partial o-projection
  (float32r -- the orthogonal-correction step downstream amplifies error in
  the v/o path, so those matmuls need more than bf16).
  One ReduceScatter per batch (2 x 8MB fp32) hands each core 256 fully-reduced
  tokens per chunk.
  Stage 2 (token-sharded): orthogonal correction + RMS (DVE Newton rsqrt, so
  the ACT engine never switches tables mid-phase) + full SwiGLU MLP over all
  512 tokens in one pass (bf16, weights streamed once), final RMS.  Output
  shards are concatenated host-side.

SBUF layout note: logical [R, C] tensors with R > 128 are stored as
[128, (R//128)*C] with row-chunk r at columns [r*C, (r+1)*C).
"""
import sys

import numpy as np

try:
    import concourse  # noqa: F401
except ImportError:
    sys.path.insert(0, "/opt/trn_rl_repo")

import ml_dtypes

import concourse.bass as bass
import concourse.tile as tile
from concourse import bacc, mybir
from concourse.bass_utils import run_bass_kernel_spmd

BF16 = mybir.dt.bfloat16
F32 = mybir.dt.float32
F32R = mybir.dt.float32r
I32 = mybir.dt.int32
AF = mybir.ActivationFunctionType
ALU = mybir.AluOpType
AX = mybir.AxisListType
RSQRT_MAGIC = 0x5F3759DF

B, N, D = 2, 2048, 1024
H, DH = 16, 64
INTER = 2816
EPS = 1e-5
T = B * N             # 4096
NC = 8
DHC = (H // NC) * DH  # 128
HC = H // NC          # 2
CHUNK = N // NC       # 256
SHARD = B * CHUNK     # 512
NB = 512
MT = N // 128         # 16
DD = D // 128         # 8
IT = INTER // 128     # 22
TQ = 1024


def _build(stage=3):
    nc = bacc.Bacc("TRN2", target_bir_lowering=False, debug=False,
                   num_devices=NC)

    hsT = nc.declare_dram_parameter("hsT", [D, T], F32R, isOutput=False)
    wqkvT = nc.declare_dram_parameter("wqkvT", [D, 3 * DHC], F32R, isOutput=False)
    cosT2 = nc.declare_dram_parameter("cosT2", [DHC, N], F32, isOutput=False)
    sinT2 = nc.declare_dram_parameter("sinT2", [DHC, N], F32, isOutput=False)
    woT = nc.declare_dram_parameter("woT", [DHC, D], F32R, isOutput=False)
    hs_sh = nc.declare_dram_parameter("hs_sh", [SHARD, D], F32, isOutput=False)
    wguT = nc.declare_dram_parameter("wguT", [D, 2 * INTER], BF16, isOutput=False)
    wdT = nc.declare_dram_parameter("wdT", [INTER, D], BF16, isOutput=False)
    ident = nc.declare_dram_parameter("ident", [128, 128], BF16, isOutput=False)
    identf = nc.declare_dram_parameter("identf", [128, 128], F32, isOutput=False)
    ones64 = nc.declare_dram_parameter("ones64", [1, 64], BF16, isOutput=False)
    blkA = nc.declare_dram_parameter("blkA", [128, 2], BF16, isOutput=False)
    blkB = nc.declare_dram_parameter("blkB", [2, 128], BF16, isOutput=False)
    out_ext = nc.declare_dram_parameter("out", [SHARD, D], F32, isOutput=True)

    rs_in = [nc.dram_tensor(f"rs_in{k}", [N, D], F32) for k in range(B)]
    rs_out = [nc.dram_tensor(f"rs_out{k}", [CHUNK, D], F32) for k in range(B)]

    hsT_r = hsT.ap().rearrange("(c p) t -> p c t", p=128)
    wqkvT_r = wqkvT.ap().rearrange("(c p) o -> p c o", p=128)
    wguT_r = wguT.ap().rearrange("(c p) o -> p c o", p=128)
    wdT_r = wdT.ap().rearrange("(c p) o -> p c o", p=128)

    with tile.TileContext(nc, num_cores=NC) as tc:
        with (
            tc.tile_pool(name="cons", bufs=1) as cons,
            tc.tile_pool(name="acts", bufs=1) as acts,
        ):
            # ---------------- constants ----------------
            c_id = cons.tile([128, 128], BF16, tag="ident")
            nc.sync.dma_start(c_id[:], ident[:])
            c_idf = cons.tile([128, 128], F32, tag="identf")
            nc.sync.dma_start(c_idf[:], identf[:])
            c_ones64 = cons.tile([1, 64], BF16, tag="ones64")
            nc.sync.dma_start(c_ones64[:], ones64[:])
            c_blkA = cons.tile([128, 2], BF16, tag="blkA")
            nc.sync.dma_start(c_blkA[:], blkA[:])
            c_blkB = cons.tile([2, 128], BF16, tag="blkB")
            nc.sync.dma_start(c_blkB[:], blkB[:])
            c_cos = cons.tile([DHC, N], F32, tag="cos")
            nc.sync.dma_start(c_cos[:], cosT2[:])
            c_sin = cons.tile([DHC, N], F32, tag="sin")
            nc.sync.dma_start(c_sin[:], sinT2[:])
            c_wqkv = cons.tile([128, DD * 3 * DHC], F32R, tag="wqkv")
            nc.sync.dma_start(
                c_wqkv[:].rearrange("p (c o) -> p c o", c=DD), wqkvT_r[:])
            c_wo = cons.tile([DHC, D], F32R, tag="wo")
            nc.sync.dma_start(c_wo[:], woT[:])
            c_eps = cons.tile([128, 1], F32, tag="eps")
            nc.vector.memset(c_eps[:], EPS)
            c_eps64 = cons.tile([128, 1], F32, tag="eps64")
            nc.vector.memset(c_eps64[:], 64.0 * EPS)

            def emit_rsqrt(pool, pref, src, scale, bias):
                # 1/sqrt(src*scale + bias) on DVE: magic + 3 Newton steps
                ms = pool.tile([128, 1], F32, tag=pref + "_ms", name=pref + "m")
                nc.vector.tensor_scalar(ms[:], src, scale, bias,
                                        ALU.mult, ALU.add)
                ti = pool.tile([128, 1], I32, tag=pref + "_ti", name=pref + "i")
                nc.vector.tensor_scalar(ti[:], ms[:].bitcast(I32), 1, None,
                                        ALU.arith_shift_right)
                nc.vector.tensor_scalar(ti[:], ti[:], -1, RSQRT_MAGIC,
                                        ALU.mult, ALU.add)
                y = pool.tile([128, 1], F32, tag=pref + "_y", name=pref + "y")
                nc.vector.tensor_copy(y[:], ti[:].bitcast(F32))
                h = pool.tile([128, 1], F32, tag=pref + "_h", name=pref + "h")
                nc.vector.tensor_scalar(h[:], ms[:], 0.5, None, ALU.mult)
                t = pool.tile([128, 1], F32, tag=pref + "_t", name=pref + "t")
                for _ in range(3):
                    nc.vector.tensor_tensor(t[:], y[:], y[:], ALU.mult)
                    nc.vector.tensor_tensor(t[:], t[:], h[:], ALU.mult)
                    nc.vector.tensor_scalar(t[:], t[:], -1.0, 1.5,
                                            ALU.mult, ALU.add)
                    nc.vector.tensor_tensor(y[:], y[:], t[:], ALU.mult)
                return y

            def wqkv_sl(dd, o0, o1):
                return c_wqkv[:, dd * 3 * DHC + o0:dd * 3 * DHC + o1]

            sb_qh = acts.tile([DHC, T], BF16, tag="qh")
            sb_kh = acts.tile([DHC, T], BF16, tag="kh")
            sb_vT = acts.tile([DHC, T], F32, tag="vT")
            sb_va = acts.tile([128, B * HC * MT * 65], BF16, tag="vaug")
            nc.vector.memset(sb_va[:], 1.0)

            def vaug_sl(b, h2, mt):
                off = ((b * HC + h2) * MT + mt) * 65
                return sb_va[:, off:off + 65]

            # ================ phase A: qkv ================
            with (
                tc.tile_pool(name="hsq", bufs=2) as hsq,
                tc.tile_pool(name="ropew", bufs=2) as rw,
                tc.tile_pool(name="pA", bufs=2, space="PSUM") as pA,
                tc.tile_pool(name="pA1", bufs=1, space="PSUM") as pA1,
            ):
                for tq in range(T // TQ):
                    hst = hsq.tile([128, DD * TQ], F32R, tag="hsT")
                    nc.sync.dma_start(
                        hst[:].rearrange("p (c t) -> p c t", c=DD),
                        hsT_r[:, :, tq * TQ:(tq + 1) * TQ])

                    for ot in range(3):
                        for j in range(TQ // 512):
                            t0 = tq * TQ + j * 512
                            col = t0 % N
                            b = t0 // N
                            pq = pA.tile([128, 512], F32, tag="qkv")
                            for dd in range(DD):
                                nc.tensor.matmul(
                                    pq[:],
                                    wqkv_sl(dd, ot * 128, (ot + 1) * 128),
                                    hst[:, dd * TQ + j * 512:
                                        dd * TQ + (j + 1) * 512],
                                    start=(dd == 0), stop=(dd == DD - 1))
                            if ot < 2:
                                t2 = rw.tile([128, 512], F32, tag="rope_t2")
                                for (dr, sr) in ((0, 32), (32, 0),
                                                 (64, 96), (96, 64)):
                                    nc.vector.tensor_tensor(
                                        t2[dr:dr + 32, :], pq[sr:sr + 32, :],
                                        c_sin[dr:dr + 32, col:col + 512],
                                        ALU.mult)
                                tq_t = rw.tile([128, 512], F32, tag="rope_q")
                                nc.vector.tensor_tensor(
                                    tq_t[:], pq[:], c_cos[:, col:col + 512],
                                    ALU.mult)
                                nc.vector.tensor_tensor(
                                    tq_t[:], tq_t[:], t2[:], ALU.add)
                                sq = rw.tile([128, 512], BF16, tag="rope_sq")
                                nc.scalar.activation(sq[:], tq_t[:], AF.Square)
                                pr = pA1.tile([2, 512], F32, tag="rms")
                                nc.tensor.matmul(pr[:], c_blkA[:], sq[:],
                                                 start=True, stop=True)
                                std = rw.tile([2, 512], F32, tag="rms_std")
                                if ot == 0:   # fold 1/8 score scale into q
                                    nc.scalar.activation(
                                        std[:], pr[:], AF.Sqrt,
                                        bias=c_eps64[0:2, :], scale=1.0)
                                else:
                                    nc.scalar.activation(
                                        std[:], pr[:], AF.Sqrt,
                                        bias=c_eps[0:2, :], scale=1.0 / 64.0)
                                rin = rw.tile([2, 512], F32, tag="rms_rin")
                                nc.vector.reciprocal(rin[:], std[:])
                                rinb = rw.tile([2, 512], BF16, tag="rms_rinb")
                                nc.vector.tensor_copy(rinb[:], rin[:])
                                pb = pA1.tile([128, 512], F32, tag="bcast")
                                nc.tensor.matmul(pb[:], c_blkB[:], rinb[:],
                                                 start=True, stop=True)
                                dst = sb_qh if ot == 0 else sb_kh
                                nc.vector.tensor_tensor(
                                    dst[:, t0:t0 + 512], tq_t[:], pb[:],
                                    ALU.mult)
                            else:
                                nc.scalar.copy(sb_vT[:, t0:t0 + 512], pq[:])
                                for h2 in range(HC):
                                    for jj in range(4):
                                        mt = (col + jj * 128) // 128
                                        ptr = pA.tile([128, 64], F32,
                                                      tag="vtr")
                                        nc.tensor.transpose(
                                            ptr[:],
                                            sb_vT[h2 * 64:(h2 + 1) * 64,
                                                  t0 + jj * 128:
                                                  t0 + (jj + 1) * 128],
                                            c_idf[h2 * 64:(h2 + 1) * 64,
                                                  h2 * 64:(h2 + 1) * 64])
                                        nc.scalar.copy(
                                            vaug_sl(b, h2, mt)[:, 0:64],
                                            ptr[:])

            # ================ attention + partial o-proj ================
            with (
                tc.tile_pool(name="attw", bufs=2) as aw,
                tc.tile_pool(name="attp", bufs=3) as ap3,
                tc.tile_pool(name="pS", bufs=2, space="PSUM") as pS,
                tc.tile_pool(name="pC", bufs=2, space="PSUM") as pC,
                tc.tile_pool(name="pB1", bufs=1, space="PSUM") as pB1,
                tc.tile_pool(name="pO", bufs=2, space="PSUM") as pO,
            ):
                for b in range(B):
                    for nb in range(N // NB):
                        n0 = b * N + nb * NB
                        mTt = ap3.tile([128, NB], F32R, tag="mT")
                        pcs = []
                        for h2 in range(HC):
                            rows = slice(h2 * 64, (h2 + 1) * 64)
                            pc = pC.tile([65, NB], F32, tag="cacc")
                            pcs.append(pc)
                            for mt in range(MT):
                                sps = pS.tile([128, NB], F32, tag="scores")
                                nc.tensor.matmul(
                                    sps[:],
                                    sb_kh[rows, b * N + mt * 128:
                                          b * N + (mt + 1) * 128],
                                    sb_qh[rows, n0:n0 + NB],
                                    start=True, stop=True)
                                pexp = ap3.tile([128, NB], BF16, tag="pexp")
                                nc.scalar.activation(pexp[:], sps[:], AF.Exp)
                                nc.tensor.matmul(
                                    pc[:], vaug_sl(b, h2, mt), pexp[:],
                                    start=(mt == 0), stop=(mt == MT - 1))
                        # epilogue: per-head softmax normalize, joint sub
                        cts = aw.tile([128, NB], F32, tag="csb")
                        nc.scalar.copy(cts[0:64, :], pcs[0][0:64, :])
                        nc.scalar.copy(cts[64:128, :], pcs[1][0:64, :])
                        ct = aw.tile([128, NB], F32, tag="cnorm")
                        for h2 in range(HC):
                            rows = slice(h2 * 64, (h2 + 1) * 64)
                            rinv = aw.tile([1, NB], F32, tag="arinv",
                                           name=f"ri{h2}")
                            nc.vector.reciprocal(rinv[:],
                                                 pcs[h2][64:65, :])
                            rb = aw.tile([1, NB], BF16, tag="arb",
                                         name=f"rb{h2}")
                            nc.vector.tensor_copy(rb[:], rinv[:])
                            pbc = pB1.tile([64, NB], F32, tag="abc")
                            nc.tensor.matmul(pbc[:], c_ones64[:], rb[:],
                                             start=True, stop=True)
                            nc.vector.tensor_tensor(ct[rows, :], cts[rows, :],
                                                    pbc[:], ALU.mult)
                        nc.vector.tensor_tensor(
                            mTt[:], ct[:], sb_vT[:, n0:n0 + NB], ALU.subtract)
                        for j in range(NB // 128):
                            att = aw.tile([128, D], F32, tag="attn_sb")
                            for oo in range(D // 512):
                                po = pO.tile([128, 512], F32, tag="oproj")
                                nc.tensor.matmul(
                                    po[:],
                                    mTt[:, j * 128:(j + 1) * 128],
                                    c_wo[:, oo * 512:(oo + 1) * 512],
                                    start=True, stop=True)
                                nc.scalar.copy(
                                    att[:, oo * 512:(oo + 1) * 512], po[:])
                            nc.sync.dma_start(
                                rs_in[b][nb * NB + j * 128:
                                         nb * NB + (j + 1) * 128, :],
                                att[:])

                    nc.gpsimd.collective_compute(
                        "ReduceScatter", ALU.add,
                        ins=[rs_in[b].ap().opt()],
                        outs=[rs_out[b].ap().opt()],
                        replica_groups=[list(range(NC))])

            # ================ stage 2 ================
            with (
                tc.tile_pool(name="s2a", bufs=2) as s2a,
                tc.tile_pool(name="s2h", bufs=4) as s2h,
                tc.tile_pool(name="s2b", bufs=1) as s2b,
                tc.tile_pool(name="s2big", bufs=1) as s2big,
                tc.tile_pool(name="wstr", bufs=4) as wstr,
            ):
                hT_all = s2big.tile([128, DD * SHARD], BF16, tag="hT")
                h_fs = []
                with tc.tile_pool(name="pT1", bufs=2, space="PSUM") as pT1:
                    for b in range(B):
                        for tt in range(CHUNK // 128):
                            g = b * (CHUNK // 128) + tt
                            row0 = b * CHUNK + tt * 128
                            a_t = s2a.tile([128, D], F32, tag="s2_attn")
                            nc.sync.dma_start(
                                a_t[:], rs_out[b][tt * 128:(tt + 1) * 128, :])
                            x_t = s2a.tile([128, D], F32, tag="s2_hs")
                            nc.sync.dma_start(x_t[:],
                                              hs_sh[row0:row0 + 128, :])
                            prod = s2a.tile([128, D], F32, tag="s2_prod")
                            dot = s2b.tile([128, 1], F32, tag="s2_dot")
                            nc.vector.tensor_tensor(prod[:], a_t[:], x_t[:],
                                                    ALU.mult)
                            nc.vector.reduce_sum(dot[:], prod[:], axis=AX.X)
                            sc = s2b.tile([128, 1], F32, tag="s2_sc")
                            nc.vector.tensor_scalar(sc[:], dot[:], -1.0, 1.0,
                                                    ALU.mult, ALU.add)
                            htil = s2a.tile([128, D], F32, tag="s2_htil")
                            nc.vector.tensor_scalar(htil[:], x_t[:], sc[:],
                                                    None, ALU.mult)
                            nc.vector.tensor_tensor(htil[:], htil[:], a_t[:],
                                                    ALU.add)
                            ssq = s2b.tile([128, 1], F32, tag="s2_ssq")
                            nc.vector.tensor_tensor(prod[:], htil[:], htil[:],
                                                    ALU.mult)
                            nc.vector.reduce_sum(ssq[:], prod[:], axis=AX.X)
                            rstd = emit_rsqrt(s2b, "n1", ssq[:], 1.0 / D, EPS)
                            h_f = s2h.tile([128, D], F32, tag="s2_h")
                            h_fs.append(h_f)
                            nc.vector.tensor_scalar(h_f[:], htil[:], rstd[:],
                                                    None, ALU.mult)
                            h_b = s2a.tile([128, D], BF16, tag="s2_hb")
                            nc.vector.tensor_copy(h_b[:], h_f[:])
                            for dd in range(DD):
                                ptr = pT1.tile([128, 128], BF16, tag="htr")
                                nc.tensor.transpose(
                                    ptr[:], h_b[:, dd * 128:(dd + 1) * 128],
                                    c_id[:])
                                nc.scalar.copy(
                                    hT_all[:, dd * SHARD + g * 128:
                                           dd * SHARD + (g + 1) * 128],
                                    ptr[:])

                actT = s2big.tile([128, IT * SHARD], BF16, tag="actT")
                with tc.tile_pool(name="pM", bufs=2, space="PSUM") as pM:
                    for it in range(IT):
                        wg = wstr.tile([128, DD * 128], BF16, tag="wg")
                        nc.sync.dma_start(
                            wg[:].rearrange("p (c o) -> p c o", c=DD),
                            wguT_r[:, :, it * 128:(it + 1) * 128])
                        wu = wstr.tile([128, DD * 128], BF16, tag="wu")
                        nc.sync.dma_start(
                            wu[:].rearrange("p (c o) -> p c o", c=DD),
                            wguT_r[:, :, INTER + it * 128:
                                   INTER + (it + 1) * 128])
                        pg = pM.tile([128, SHARD], F32, tag="gate")
                        pu = pM.tile([128, SHARD], F32, tag="up")
                        for dd in range(DD):
                            nc.tensor.matmul(
                                pg[:], wg[:, dd * 128:(dd + 1) * 128],
                                hT_all[:, dd * SHARD:(dd + 1) * SHARD],
                                start=(dd == 0), stop=(dd == DD - 1))
                        for dd in range(DD):
                            nc.tensor.matmul(
                                pu[:], wu[:, dd * 128:(dd + 1) * 128],
                                hT_all[:, dd * SHARD:(dd + 1) * SHARD],
                                start=(dd == 0), stop=(dd == DD - 1))
                        sg = s2a.tile([128, SHARD], F32, tag="s2_silu")
                        nc.scalar.activation(sg[:], pg[:], AF.Silu)
                        nc.vector.tensor_tensor(
                            actT[:, it * SHARD:(it + 1) * SHARD], sg[:],
                            pu[:], ALU.mult)

                with tc.tile_pool(name="pD", bufs=4, space="PSUM") as pD:
                    for gg in range(SHARD // 256):   # two 256-token groups
                        pms = []
                        for g2 in range(2):
                            pm0 = pD.tile([128, 512], F32, tag="dn")
                            pm1 = pD.tile([128, 512], F32, tag="dn")
                            pms.append((pm0, pm1))
                        for it in range(IT):
                            wd = wstr.tile([128, D], BF16, tag="wd")
                            nc.sync.dma_start(wd[:], wdT_r[:, it, :])
                            for g2 in range(2):
                                g = gg * 2 + g2
                                lhs = actT[:, it * SHARD + g * 128:
                                           it * SHARD + (g + 1) * 128]
                                nc.tensor.matmul(
                                    pms[g2][0][:], lhs, wd[:, 0:512],
                                    start=(it == 0), stop=(it == IT - 1))
                                nc.tensor.matmul(
                                    pms[g2][1][:], lhs, wd[:, 512:1024],
                                    start=(it == 0), stop=(it == IT - 1))
                        for g2 in range(2):
                            g = gg * 2 + g2
                            b, tt = divmod(g, CHUNK // 128)
                            row0 = b * CHUNK + tt * 128
                            h_f = h_fs[g]
                            o1 = s2a.tile([128, D], F32, tag="s2_o1")
                            nc.vector.tensor_tensor(
                                o1[:, 0:512], h_f[:, 0:512], pms[g2][0][:],
                                ALU.add)
                            nc.vector.tensor_tensor(
                                o1[:, 512:1024], h_f[:, 512:1024],
                                pms[g2][1][:], ALU.add)
                            prod2 = s2a.tile([128, D], F32, tag="s2_prod2")
                            ssq2 = s2b.tile([128, 1], F32, tag="s2_ssq2")
                            nc.vector.tensor_tensor(prod2[:], o1[:], o1[:],
                                                    ALU.mult)
                            nc.vector.reduce_sum(ssq2[:], prod2[:], axis=AX.X)
                            rstd2 = emit_rsqrt(s2b, "n2", ssq2[:],
                                               1.0 / D, EPS)
                            o_f = s2a.tile([128, D], F32, tag="s2_out")
                            nc.vector.tensor_scalar(o_f[:], o1[:], rstd2[:],
                                                    None, ALU.mult)
                            nc.sync.dma_start(out_ext[row0:row0 + 128, :],
                                              o_f[:])

    nc.compile()
    return nc


_CACHE = {}


def _prep_inputs(inputs):
    hs = np.ascontiguousarray(inputs["hidden_states"], np.float32)
    cos = np.ascontiguousarray(inputs["cos"], np.float32)
    sin = np.ascontiguousarray(inputs["sin"], np.float32)
    w_qkv = np.ascontiguousarray(inputs["w_qkv"], np.float32)
    w_o = np.ascontiguousarray(inputs["w_o"], np.float32)
    w_gu = np.ascontiguousarray(inputs["w_gate_up"], np.float32)
    w_dn = np.ascontiguousarray(inputs["w_down"], np.float32)

    bf = ml_dtypes.bfloat16
    hs_flat = hs.reshape(T, D)
    hsT = np.ascontiguousarray(hs_flat.T)

    cosT = cos.T
    sinTn = sin.T.copy()
    sinTn[:32] *= -1.0
    cosT2 = np.ascontiguousarray(np.concatenate([cosT, cosT], 0), np.float32)
    sinT2 = np.ascontiguousarray(np.concatenate([sinTn, sinTn], 0), np.float32)

    ident = np.eye(128, dtype=bf)
    identf = np.eye(128, dtype=np.float32)
    ones64 = np.ones((1, 64), dtype=bf)
    blkA = np.zeros((128, 2), dtype=bf)
    blkA[0:64, 0] = 1
    blkA[64:128, 1] = 1
    blkB = np.zeros((2, 128), dtype=bf)
    blkB[0, 0:64] = 1
    blkB[1, 64:128] = 1

    wguT = np.ascontiguousarray(w_gu.T).astype(bf)
    wdT = np.ascontiguousarray(w_dn.T).astype(bf)

    in_maps = []
    for c in range(NC):
        r0 = c * DHC
        wq = w_qkv[r0:r0 + DHC]
        wk = w_qkv[D + r0:D + r0 + DHC]
        wv = w_qkv[2 * D + r0:2 * D + r0 + DHC]
        wqkvT_c = np.ascontiguousarray(np.concatenate([wq, wk, wv], 0).T)
        woT_c = np.ascontiguousarray(w_o[:, r0:r0 + DHC].T)
        tok = np.concatenate([np.arange(CHUNK * c, CHUNK * (c + 1)),
                              N + np.arange(CHUNK * c, CHUNK * (c + 1))])
        in_maps.append({
            "hsT": hsT, "wqkvT": wqkvT_c, "cosT2": cosT2, "sinT2": sinT2,
            "woT": woT_c, "hs_sh": np.ascontiguousarray(hs_flat[tok]),
            "wguT": wguT, "wdT": wdT, "ident": ident, "identf": identf,
            "ones64": ones64, "blkA": blkA, "blkB": blkB,
        })
    return in_maps


def run_sharded(inputs, trace=False):
    if "nc" not in _CACHE:
        _CACHE["nc"] = _build()
    nc = _CACHE["nc"]
    in_maps = _prep_inputs(inputs)
    res = run_bass_kernel_spmd(nc, in_maps, core_ids=list(range(NC)),
                               trace=trace)
    out_full = np.empty((T, D), np.float32)
    for c in range(NC):
        shard = np.asarray(res.results[c]["out"], np.float32)
        out_full[CHUNK * c:CHUNK * (c + 1)] = shard[:CHUNK]
        out_full[N + CHUNK * c:N + CHUNK * (c + 1)] = shard[CHUNK:]
    return out_full.reshape(B, N, D), res


def kernel(**inputs):
    out, _ = run_sharded(inputs, trace=False)
    return out
